# revision 17
# baseline (speedup 1.0000x reference)
"""Cross-attention kernel for one TRN2 chip (8 NeuronCores), Bass/Tile.

Math (matches the reference nn.Module):
    q = src @ Wq.T          (B, S, H)
    k = tar @ Wk.T          (B, T, H)
    v = ori_src @ Wv.T      (B, S, H)
    scores[b,t,s] = q[b,s] . k[b,t] / sqrt(H)
    attn = softmax(scores, axis=-1)             # over s
    context = attn @ v                          (B, T, H)
    returns (context, attn)

Sharding: pure data parallel over (batch, t-half). Core c handles batch
c//2 and T rows [(c%2)*1024, (c%2+1)*1024). Weights are replicated.
Softmax rows are never split, so no collectives are needed.

Per-core kernel layout choices (P=128 partitions):
  - activations are fed pre-transposed [H, seq] so the contraction dim (h)
    lands on partitions without any on-chip transpose;
  - qT[h',s], kT[h',t] are produced directly in transposed layout so the
    scores matmul contracts h' on partitions; v[s,h''] is produced in
    natural layout for the PV matmul;
  - softmax runs over the free axis; the probabilities are transposed
    128x128 via the PE (bf16, 1 cyc/row) for the PV contraction over s;
  - 1/sqrt(H)=1/32 is folded into qT (exact in bf16); 1/rowsum is folded
    into the context epilogue; attn output is normalized fp32.

All matmuls are bf16 with fp32 PSUM accumulation.
"""

import numpy as np
import ml_dtypes

import concourse.bass as bass
import concourse.tile as tile
from concourse import bacc, mybir
from concourse.bass import ds
from concourse.masks import make_identity

B, T, S, H = 4, 2048, 2048, 1024
NCORES = 8
TSH = T // 2  # t rows per core
P = 128
F32 = mybir.dt.float32
BF16 = mybir.dt.bfloat16
HT = H // P  # 8 contraction tiles along h / h'
ST = S // P  # 16 s tiles
TT = TSH // P  # 8 t tiles per core
SC = S // 512  # 4 s chunks
TC = TSH // 512  # 2 t chunks
HC = H // 512  # 2 h chunks

_NC_CACHE: dict[tuple, "bacc.Bacc"] = {}
_JIT_CACHE: dict[tuple, object] = {}

# Pair-split: cores 2b and 2b+1 each compute q/v projections for half of S,
# then AllGather within the pair — removes the duplicated projection FLOPs.
PAIR_SPLIT = True
RG = [[0, 1], [2, 3], [4, 5], [6, 7]]


def _emit_body(tc, pools, aps, pair_split):
    """Emit one full forward pass for this core."""
    nc = tc.nc
    (p_w, p_act, p_qkv, p_p32, p_p16, p_pt, p_stat, p_cx, p_dram, ps_mm, ps_tp) = pools
    (srcT_r, tarT_r, oriT_r, wqT_r, wkT_r, wvT_r, ctx_d, attn_d, ident) = aps

    EXP = mybir.ActivationFunctionType.Exp
    AX = mybir.AxisListType.X

    def load_w(w_r):
        """Load one [H, H] weight in 4 pieces so first matmuls start early."""
        w = p_w.tile([P, HT, H], BF16, tag="w")
        for piece in range(4):
            nc.sync.dma_start(
                w[:, ds(piece * 2, 2), :], w_r[:, ds(piece * 2, 2), :]
            )
        return w

    def proj_seq(w, act_r, n_chunks, out_sb, scale=None):
        """out_sb[h' tile m, x] = W.T @ act  over x chunks of 512."""
        for c in range(n_chunks):
            a = p_act.tile([P, HT, 512], BF16, tag="act")
            for piece in range(2):
                nc.sync.dma_start(
                    a[:, ds(piece * 4, 4), :],
                    act_r[:, ds(piece * 4, 4), ds(c * 512, 512)],
                )
            for m in range(HT):
                ps = ps_mm.tile([P, 512], F32, tag="mm")
                for k in range(HT):
                    nc.tensor.matmul(
                        ps[:],
                        lhsT=w[:, k, ds(m * P, P)],
                        rhs=a[:, k, :],
                        start=(k == 0),
                        stop=(k == HT - 1),
                    )
                if scale is None:
                    nc.any.tensor_copy(out=out_sb[:, m, ds(c * 512, 512)], in_=ps[:])
                else:
                    nc.any.tensor_scalar_mul(
                        out_sb[:, m, ds(c * 512, 512)], ps[:], scale
                    )

    def proj_v(w, act_r, n_chunks, out_sb, so_base):
        """out_sb[s tile, h''] = act.T @ W.T over s chunks of 512."""
        for c in range(n_chunks):
            a = p_act.tile([P, HT, 512], BF16, tag="act")
            nc.sync.dma_start(a[:], act_r[:, :, ds(c * 512, 512)])
            for sl in range(4):
                so = so_base + c * 4 + sl
                for hc in range(HC):
                    ps = ps_mm.tile([P, 512], F32, tag="mm")
                    for k in range(HT):
                        nc.tensor.matmul(
                            ps[:],
                            lhsT=a[:, k, ds(sl * P, P)],
                            rhs=w[:, k, ds(hc * 512, 512)],
                            start=(k == 0),
                            stop=(k == HT - 1),
                        )
                    nc.any.tensor_copy(out=out_sb[:, so, ds(hc * 512, 512)], in_=ps[:])

    if pair_split:
        # Each core projects only its S-half of q and v, pairs AllGather.
        wq = load_w(wqT_r)
        qh = p_qkv.tile([P, HT, TSH], BF16, tag="half")
        proj_seq(wq, srcT_r, TC, qh, scale=1.0 / 32.0)
        qh_d = p_dram.tile([P, HT, TSH], BF16, tag="qh_d")
        qg_d = p_dram.tile([2, P, HT, TSH], BF16, tag="qg_d")
        nc.sync.dma_start(qh_d[:], qh[:])
        nc.gpsimd.collective_compute(
            "AllGather",
            mybir.AluOpType.bypass,
            replica_groups=RG,
            ins=[qh_d[:]],
            outs=[qg_d[:]],
        )
        qT = p_qkv.tile([P, HT, S], BF16, tag="qT")
        for i in range(2):
            nc.sync.dma_start(qT[:, :, ds(i * TSH, TSH)], qg_d[i])

        wv = load_w(wvT_r)
        vh = p_qkv.tile([P, HT, H], BF16, tag="half")
        proj_v(wv, oriT_r, TC, vh, so_base=0)
        vh_d = p_dram.tile([P, HT, H], BF16, tag="vh_d")
        vg_d = p_dram.tile([2, P, HT, H], BF16, tag="vg_d")
        nc.sync.dma_start(vh_d[:], vh[:])
        nc.gpsimd.collective_compute(
            "AllGather",
            mybir.AluOpType.bypass,
            replica_groups=RG,
            ins=[vh_d[:]],
            outs=[vg_d[:]],
        )
        v = p_qkv.tile([P, ST, H], BF16, tag="v")
        for i in range(2):
            nc.sync.dma_start(v[:, ds(i * HT, HT), :], vg_d[i])

        wk = load_w(wkT_r)
        kT = p_qkv.tile([P, HT, TSH], BF16, tag="kT")
        proj_seq(wk, tarT_r, TC, kT)
    else:
        kT = p_qkv.tile([P, HT, TSH], BF16, tag="kT")
        wk = load_w(wkT_r)
        proj_seq(wk, tarT_r, TC, kT)
        qT = p_qkv.tile([P, HT, S], BF16, tag="qT")
        wq = load_w(wqT_r)
        proj_seq(wq, srcT_r, SC, qT, scale=1.0 / 32.0)
        v = p_qkv.tile([P, ST, H], BF16, tag="v")
        wv = load_w(wvT_r)
        proj_v(wv, oriT_r, SC, v, so_base=0)

    # ---- attention, software-pipelined over 128-row t tiles:
    # PE order is scores(0), [scores(1), tail(0)], [scores(2), tail(1)], ...
    # so the softmax latency chain of tile t hides under scores(t+1).
    def emit_scores(t):
        mx4 = p_stat.tile([P, SC], F32, tag="mx4")
        pss = []
        for c in range(SC):
            ps = ps_mm.tile([P, 512], F32, tag="mm")
            for k in range(HT):
                nc.tensor.matmul(
                    ps[:],
                    lhsT=kT[:, k, ds(t * P, P)],
                    rhs=qT[:, k, ds(c * 512, 512)],
                    start=(k == 0),
                    stop=(k == HT - 1),
                )
            nc.vector.reduce_max(mx4[:, ds(c, 1)], ps[:], axis=AX)
            pss.append(ps)
        negmx = p_stat.tile([P, 1], F32, tag="negmx")
        nc.vector.reduce_max(negmx[:], mx4[:], axis=AX, negate=True)
        return pss, negmx

    def emit_tail(t, pss, negmx):
        # exp(score - max) in fp32 (+ row-sums), bf16 copy for the PV matmul
        p32 = p_p32.tile([P, S], F32, tag="p32")
        p16 = p_p16.tile([P, S], BF16, tag="p16")
        sums = p_stat.tile([P, SC], F32, tag="sums")
        for c in range(SC):
            nc.scalar.activation(
                p32[:, ds(c * 512, 512)],
                pss[c][:],
                EXP,
                bias=negmx[:],
                scale=1.0,
                accum_out=sums[:, ds(c, 1)],
            )
            nc.vector.tensor_copy(
                out=p16[:, ds(c * 512, 512)], in_=p32[:, ds(c * 512, 512)]
            )
        ssum = p_stat.tile([P, 1], F32, tag="ssum")
        nc.vector.reduce_sum(ssum[:], sums[:], axis=AX)
        inv = p_stat.tile([P, 1], F32, tag="inv")
        nc.vector.reciprocal(inv[:], ssum[:])

        # transpose P (unnormalized, bf16) 128x128 via PE: pt[s, t]
        pt = p_pt.tile([P, S], BF16, tag="pt")
        for g in range(SC):
            pst = ps_tp.tile([P, 512], BF16, tag="tp")
            for j in range(4):
                sblk = g * 4 + j
                nc.tensor.transpose(
                    pst[:, ds(j * P, P)], p16[:, ds(sblk * P, P)], ident[:]
                )
            nc.scalar.copy(pt[:, ds(g * 512, 512)], pst[:])

        # PV: ctx[t, h''] = sum_s P[t,s] v[s,h''] * inv
        for hc in range(HC):
            psc = ps_mm.tile([P, 512], F32, tag="mm")
            for si in range(ST):
                nc.tensor.matmul(
                    psc[:],
                    lhsT=pt[:, ds(si * P, P)],
                    rhs=v[:, si, ds(hc * 512, 512)],
                    start=(si == 0),
                    stop=(si == ST - 1),
                )
            cxs = p_cx.tile([P, 512], F32, tag="cxs")
            nc.vector.tensor_scalar_mul(cxs[:], psc[:], inv[:])
            nc.sync.dma_start(ctx_d[ds(t * P, P), ds(hc * 512, 512)], cxs[:])

        # normalized fp32 attention rows out
        nc.vector.tensor_scalar_mul(p32[:], p32[:], inv[:])
        nc.sync.dma_start(attn_d[ds(t * P, P), :], p32[:])

    pending = emit_scores(0)
    for t in range(TT):
        nxt = emit_scores(t + 1) if t + 1 < TT else None
        emit_tail(t, *pending)
        pending = nxt


def build(repeat: int = 1, pair_split: bool | None = None) -> "bacc.Bacc":
    if pair_split is None:
        pair_split = PAIR_SPLIT
    nc = bacc.Bacc(
        "TRN2",
        target_bir_lowering=False,
        debug=False,
        enable_asserts=False,
        num_devices=NCORES,
    )
    s_in = TSH if pair_split else S  # q/v activations arrive halved if split
    srcT = nc.dram_tensor("srcT", [H, s_in], BF16, kind="ExternalInput").ap()
    tarT = nc.dram_tensor("tarT", [H, TSH], BF16, kind="ExternalInput").ap()
    oriT = nc.dram_tensor("oriT", [H, s_in], BF16, kind="ExternalInput").ap()
    wqT = nc.dram_tensor("wqT", [H, H], BF16, kind="ExternalInput").ap()
    wkT = nc.dram_tensor("wkT", [H, H], BF16, kind="ExternalInput").ap()
    wvT = nc.dram_tensor("wvT", [H, H], BF16, kind="ExternalInput").ap()
    ctx_d = nc.dram_tensor("ctx", [TSH, H], F32, kind="ExternalOutput").ap()
    attn_d = nc.dram_tensor("attn", [TSH, S], F32, kind="ExternalOutput").ap()

    # [h, x] -> [p, ho, x] with h = ho*P + p (partition-inner)
    srcT_r = srcT.rearrange("(ko p) s -> p ko s", p=P)
    tarT_r = tarT.rearrange("(ko p) t -> p ko t", p=P)
    oriT_r = oriT.rearrange("(ko p) s -> p ko s", p=P)
    wqT_r = wqT.rearrange("(ko p) m -> p ko m", p=P)
    wkT_r = wkT.rearrange("(ko p) m -> p ko m", p=P)
    wvT_r = wvT.rearrange("(ko p) m -> p ko m", p=P)

    with tile.TileContext(nc) as tc:
        with (
            tc.tile_pool(name="w", bufs=2) as p_w,
            tc.tile_pool(name="act", bufs=3) as p_act,
            tc.tile_pool(name="qkv", bufs=1) as p_qkv,
            tc.tile_pool(name="p32", bufs=2) as p_p32,
            tc.tile_pool(name="p16", bufs=2) as p_p16,
            tc.tile_pool(name="pt", bufs=2) as p_pt,
            tc.tile_pool(name="stat", bufs=3) as p_stat,
            tc.tile_pool(name="cxs", bufs=3) as p_cx,
            tc.tile_pool(name="cst", bufs=1) as p_cst,
            tc.tile_pool(name="dram", bufs=1, space="DRAM") as p_dram,
            tc.tile_pool(name="mm", bufs=6, space="PSUM") as ps_mm,
            tc.tile_pool(name="tp", bufs=2, space="PSUM") as ps_tp,
        ):
            ident = p_cst.tile([P, P], BF16, tag="ident")
            make_identity(nc, ident[:])
            pools = (
                p_w, p_act, p_qkv, p_p32, p_p16, p_pt, p_stat, p_cx, p_dram,
                ps_mm, ps_tp,
            )
            aps = (srcT_r, tarT_r, oriT_r, wqT_r, wkT_r, wvT_r, ctx_d, attn_d, ident)
            for _ in range(repeat):
                _emit_body(tc, pools, aps, pair_split)
    nc.compile()
    return nc


def get_nc(repeat: int = 1, pair_split: bool | None = None) -> "bacc.Bacc":
    if pair_split is None:
        pair_split = PAIR_SPLIT
    key = (repeat, pair_split)
    if key not in _NC_CACHE:
        _NC_CACHE[key] = build(repeat, pair_split)
    return _NC_CACHE[key]


def shard_inputs(tar, src, ori_src, Wq, Wk, Wv):
    """Host-side layout work only: transpose / cast / slice (no math)."""
    bf = ml_dtypes.bfloat16
    f32 = np.float32
    tar = np.asarray(tar, f32)
    src = np.asarray(src, f32)
    ori_src = np.asarray(ori_src, f32)
    wqT = np.ascontiguousarray(np.asarray(Wq, f32).T).astype(bf)
    wkT = np.ascontiguousarray(np.asarray(Wk, f32).T).astype(bf)
    wvT = np.ascontiguousarray(np.asarray(Wv, f32).T).astype(bf)
    srcT = np.ascontiguousarray(src.transpose(0, 2, 1)).astype(bf)  # [B, H, S]
    oriT = np.ascontiguousarray(ori_src.transpose(0, 2, 1)).astype(bf)
    tarT = np.ascontiguousarray(tar.transpose(0, 2, 1)).astype(bf)  # [B, H, T]

    in_maps = []
    for c in range(NCORES):
        b, th = divmod(c, 2)
        sl = slice(th * TSH, (th + 1) * TSH)
        in_maps.append(
            {
                "srcT": np.ascontiguousarray(srcT[b][:, sl]) if PAIR_SPLIT else srcT[b],
                "tarT": np.ascontiguousarray(tarT[b][:, sl]),
                "oriT": np.ascontiguousarray(oriT[b][:, sl]) if PAIR_SPLIT else oriT[b],
                "wqT": wqT,
                "wkT": wkT,
                "wvT": wvT,
            }
        )
    return in_maps


class Dispatcher:
    """Compile-once / run-many PJRT dispatch for one Bass program.

    Mirrors concourse.bass2jax.run_bass_via_pjrt but caches the jitted
    callable and keeps inputs resident on device, so repeated runs do not
    pay re-trace/re-compile or host->device transfer.
    """

    def __init__(self, nc):
        import jax
        from jax.sharding import Mesh, PartitionSpec
        from jax.experimental.shard_map import shard_map
        from concourse import bass2jax

        bass2jax.install_neuronx_cc_hook()
        self._jax = jax
        self.nc = nc
        in_names, out_names, out_avals, zero_outs = [], [], [], []
        partition_name = (
            nc.partition_id_tensor.name if nc.partition_id_tensor else None
        )
        for alloc in nc.m.functions[0].allocations:
            if not isinstance(alloc, mybir.MemoryLocationSet):
                continue
            name = alloc.memorylocations[0].name
            if alloc.kind == "ExternalInput":
                if name != partition_name:
                    in_names.append(name)
            elif alloc.kind == "ExternalOutput":
                shape = tuple(alloc.tensor_shape)
                dtype = mybir.dt.np(alloc.dtype)
                out_names.append(name)
                out_avals.append(jax.core.ShapedArray(shape, dtype))
                zero_outs.append(np.zeros(shape, dtype))
        self.in_names = list(in_names)
        self.out_names = out_names
        self.out_avals = out_avals
        self.zero_outs = zero_outs
        n_params = len(in_names)
        all_in_names = in_names + out_names
        if partition_name is not None:
            all_in_names.append(partition_name)

        from concourse.bass2jax import _bass_exec_p, partition_id_tensor

        def _body(*args):
            operands = list(args)
            if partition_name is not None:
                operands.append(partition_id_tensor())
            outs = _bass_exec_p.bind(
                *operands,
                out_avals=tuple(out_avals),
                in_names=tuple(all_in_names),
                out_names=tuple(out_names),
                lowering_input_output_aliases=(),
                sim_require_finite=True,
                sim_require_nnan=True,
                nc=nc,
            )
            return tuple(outs)

        devices = jax.devices()[:NCORES]
        mesh = Mesh(np.asarray(devices), ("core",))
        n_all = n_params + len(out_names)
        self._fn = jax.jit(
            shard_map(
                _body,
                mesh=mesh,
                in_specs=(PartitionSpec("core"),) * n_all,
                out_specs=(PartitionSpec("core"),) * len(out_names),
                check_rep=False,
            ),
            keep_unused=True,
        )
        self._dev_zero = None

    def put(self, in_maps):
        """Concat per-core inputs on axis 0 and move to device once."""
        jax = self._jax
        concat = [
            np.concatenate([np.asarray(m[name]) for m in in_maps], axis=0)
            for name in self.in_names
        ]
        dev_in = [jax.device_put(a) for a in concat]
        if self._dev_zero is None:
            self._dev_zero = [
                jax.device_put(
                    np.zeros((NCORES * z.shape[0], *z.shape[1:]), z.dtype)
                )
                for z in self.zero_outs
            ]
        return dev_in

    def run(self, dev_in):
        outs = self._fn(*dev_in, *self._dev_zero)
        self._jax.block_until_ready(outs)
        return outs

    def run_host(self, in_maps):
        outs = self.run(self.put(in_maps))
        per_core = []
        for c in range(NCORES):
            d = {}
            for i, name in enumerate(self.out_names):
                full = np.asarray(outs[i])
                per_shard = self.out_avals[i].shape
                d[name] = full.reshape(NCORES, *per_shard)[c]
            per_core.append(d)
        return per_core


def get_dispatcher(repeat: int = 1) -> Dispatcher:
    key = (repeat, PAIR_SPLIT)
    if key not in _JIT_CACHE:
        _JIT_CACHE[key] = Dispatcher(get_nc(repeat))
    return _JIT_CACHE[key]


def kernel(tar, src, ori_src, Wq, Wk, Wv):
    in_maps = shard_inputs(tar, src, ori_src, Wq, Wk, Wv)
    results = get_dispatcher().run_host(in_maps)
    context = np.empty((B, T, H), np.float32)
    attn = np.empty((B, T, S), np.float32)
    for c in range(NCORES):
        b, th = divmod(c, 2)
        context[b, th * TSH : (th + 1) * TSH] = results[c]["ctx"]
        attn[b, th * TSH : (th + 1) * TSH] = results[c]["attn"]
    return context, attn


# revision 24
# speedup vs baseline: 1.1167x; 1.1167x over previous
"""Cross-attention kernel for one TRN2 chip (8 NeuronCores), Bass/Tile.

Math (matches the reference nn.Module):
    q = src @ Wq.T          (B, S, H)
    k = tar @ Wk.T          (B, T, H)
    v = ori_src @ Wv.T      (B, S, H)
    scores[b,t,s] = q[b,s] . k[b,t] / sqrt(H)
    attn = softmax(scores, axis=-1)             # over s
    context = attn @ v                          (B, T, H)
    returns (context, attn)

Sharding: pure data parallel over (batch, t-half). Core c handles batch
c//2 and T rows [(c%2)*1024, (c%2+1)*1024). Weights are replicated.
Softmax rows are never split, so no collectives are needed.

Per-core kernel layout choices (P=128 partitions):
  - activations are fed pre-transposed [H, seq] so the contraction dim (h)
    lands on partitions without any on-chip transpose;
  - qT[h',s], kT[h',t] are produced directly in transposed layout so the
    scores matmul contracts h' on partitions; v[s,h''] is produced in
    natural layout for the PV matmul;
  - softmax runs over the free axis; the probabilities are transposed
    128x128 via the PE (bf16, 1 cyc/row) for the PV contraction over s;
  - 1/sqrt(H)=1/32 is folded into qT (exact in bf16); 1/rowsum is folded
    into the context epilogue; attn output is normalized fp32.

All matmuls are bf16 with fp32 PSUM accumulation.
"""

import numpy as np
import ml_dtypes

import concourse.bass as bass
import concourse.tile as tile
from concourse import bacc, mybir
from concourse.bass import ds
from concourse.masks import make_identity

B, T, S, H = 4, 2048, 2048, 1024
NCORES = 8
TSH = T // 2  # t rows per core
P = 128
F32 = mybir.dt.float32
BF16 = mybir.dt.bfloat16
HT = H // P  # 8 contraction tiles along h / h'
ST = S // P  # 16 s tiles
TT = TSH // P  # 8 t tiles per core
SC = S // 512  # 4 s chunks
TC = TSH // 512  # 2 t chunks
HC = H // 512  # 2 h chunks

_NC_CACHE: dict[tuple, "bacc.Bacc"] = {}
_JIT_CACHE: dict[tuple, object] = {}

# V-split: cores 2b and 2b+1 each compute the v projection for half of S,
# then AllGather within the pair — removes the duplicated v FLOPs. The v
# gather is scheduled first so its ~55 GB/s collective hides entirely under
# the q and k projections. (Splitting q too was measured slower: the q
# gather sits on the critical path into the scores matmuls.)
PAIR_SPLIT = True  # here meaning: v-split enabled
RG = [[0, 1], [2, 3], [4, 5], [6, 7]]


def _emit_body(tc, pools, aps, pair_split):
    """Emit one full forward pass for this core."""
    nc = tc.nc
    (p_w, p_act, p_qkv, p_p32, p_p16, p_pt, p_stat, p_cx, p_dram, ps_mm, ps_tp) = pools
    (srcT_r, tarT_r, oriT_r, wqT_r, wkT_r, wvT_r, ctx_d, attn_d, ident) = aps

    EXP = mybir.ActivationFunctionType.Exp
    AX = mybir.AxisListType.X

    def load_w(w_r):
        """Load one [H, H] weight in 4 pieces, spread over two DMA queues so
        the startup loads don't serialize on one engine's queue."""
        w = p_w.tile([P, HT, H], BF16, tag="w")
        for piece in range(4):
            eng = nc.sync if piece % 2 == 0 else nc.scalar
            eng.dma_start(w[:, ds(piece * 2, 2), :], w_r[:, ds(piece * 2, 2), :])
        return w

    def proj_seq(w, act_r, n_chunks, out_sb, scale=None):
        """out_sb[h' tile m, x] = W.T @ act  over x chunks of 512."""
        for c in range(n_chunks):
            a = p_act.tile([P, HT, 512], BF16, tag="act")
            for piece in range(2):
                nc.sync.dma_start(
                    a[:, ds(piece * 4, 4), :],
                    act_r[:, ds(piece * 4, 4), ds(c * 512, 512)],
                )
            for m in range(HT):
                ps = ps_mm.tile([P, 512], F32, tag="mm")
                for k in range(HT):
                    nc.tensor.matmul(
                        ps[:],
                        lhsT=w[:, k, ds(m * P, P)],
                        rhs=a[:, k, :],
                        start=(k == 0),
                        stop=(k == HT - 1),
                    )
                if scale is None:
                    nc.any.tensor_copy(out=out_sb[:, m, ds(c * 512, 512)], in_=ps[:])
                else:
                    nc.any.tensor_scalar_mul(
                        out_sb[:, m, ds(c * 512, 512)], ps[:], scale
                    )

    def proj_v(w, act_r, n_chunks, out_sb, so_base):
        """out_sb[s tile, h''] = act.T @ W.T over s chunks of 512."""
        for c in range(n_chunks):
            a = p_act.tile([P, HT, 512], BF16, tag="act")
            nc.sync.dma_start(a[:], act_r[:, :, ds(c * 512, 512)])
            for sl in range(4):
                so = so_base + c * 4 + sl
                for hc in range(HC):
                    ps = ps_mm.tile([P, 512], F32, tag="mm")
                    for k in range(HT):
                        nc.tensor.matmul(
                            ps[:],
                            lhsT=a[:, k, ds(sl * P, P)],
                            rhs=w[:, k, ds(hc * 512, 512)],
                            start=(k == 0),
                            stop=(k == HT - 1),
                        )
                    nc.any.tensor_copy(out=out_sb[:, so, ds(hc * 512, 512)], in_=ps[:])

    if pair_split:
        # v first: its gather chain hides under the q and k projections.
        wv = load_w(wvT_r)
        vh = p_qkv.tile([P, HT, H], BF16, tag="half")
        proj_v(wv, oriT_r, TC, vh, so_base=0)
        vh_d = p_dram.tile([P, HT, H], BF16, tag="vh_d")
        vg_d = p_dram.tile([2, P, HT, H], BF16, tag="vg_d")
        nc.sync.dma_start(vh_d[:], vh[:])
        nc.gpsimd.collective_compute(
            "AllGather",
            mybir.AluOpType.bypass,
            replica_groups=RG,
            ins=[vh_d[:]],
            outs=[vg_d[:]],
        )
        v = p_qkv.tile([P, ST, H], BF16, tag="v")
        for i in range(2):
            nc.sync.dma_start(v[:, ds(i * HT, HT), :], vg_d[i])

        qT = p_qkv.tile([P, HT, S], BF16, tag="qT")
        wq = load_w(wqT_r)
        proj_seq(wq, srcT_r, SC, qT, scale=1.0 / 32.0)

        wk = load_w(wkT_r)
        kT = p_qkv.tile([P, HT, TSH], BF16, tag="kT")
        proj_seq(wk, tarT_r, TC, kT)
    else:
        kT = p_qkv.tile([P, HT, TSH], BF16, tag="kT")
        wk = load_w(wkT_r)
        proj_seq(wk, tarT_r, TC, kT)
        qT = p_qkv.tile([P, HT, S], BF16, tag="qT")
        wq = load_w(wqT_r)
        proj_seq(wq, srcT_r, SC, qT, scale=1.0 / 32.0)
        v = p_qkv.tile([P, ST, H], BF16, tag="v")
        wv = load_w(wvT_r)
        proj_v(wv, oriT_r, SC, v, so_base=0)

    # ---- attention, software-pipelined over 128-row t tiles:
    # PE order is scores(0), [scores(1), tail(0)], [scores(2), tail(1)], ...
    # so the softmax latency chain of tile t hides under scores(t+1).
    def emit_scores(t):
        mx4 = p_stat.tile([P, SC], F32, tag="mx4")
        pss = []
        for c in range(SC):
            ps = ps_mm.tile([P, 512], F32, tag="mm")
            for k in range(HT):
                nc.tensor.matmul(
                    ps[:],
                    lhsT=kT[:, k, ds(t * P, P)],
                    rhs=qT[:, k, ds(c * 512, 512)],
                    start=(k == 0),
                    stop=(k == HT - 1),
                )
            nc.vector.reduce_max(mx4[:, ds(c, 1)], ps[:], axis=AX)
            pss.append(ps)
        negmx = p_stat.tile([P, 1], F32, tag="negmx")
        nc.vector.reduce_max(negmx[:], mx4[:], axis=AX, negate=True)
        return pss, negmx

    def emit_tail(t, pss, negmx):
        # exp(score - max) in fp32 (+ row-sums), bf16 copy for the PV matmul
        p32 = p_p32.tile([P, S], F32, tag="p32")
        p16 = p_p16.tile([P, S], BF16, tag="p16")
        sums = p_stat.tile([P, SC], F32, tag="sums")
        for c in range(SC):
            nc.scalar.activation(
                p32[:, ds(c * 512, 512)],
                pss[c][:],
                EXP,
                bias=negmx[:],
                scale=1.0,
                accum_out=sums[:, ds(c, 1)],
            )
            nc.vector.tensor_copy(
                out=p16[:, ds(c * 512, 512)], in_=p32[:, ds(c * 512, 512)]
            )
        ssum = p_stat.tile([P, 1], F32, tag="ssum")
        nc.vector.reduce_sum(ssum[:], sums[:], axis=AX)
        inv = p_stat.tile([P, 1], F32, tag="inv")
        nc.vector.reciprocal(inv[:], ssum[:])

        # transpose P (unnormalized, bf16) 128x128 via PE: pt[s, t]
        pt = p_pt.tile([P, S], BF16, tag="pt")
        for g in range(SC):
            pst = ps_tp.tile([P, 512], BF16, tag="tp")
            for j in range(4):
                sblk = g * 4 + j
                nc.tensor.transpose(
                    pst[:, ds(j * P, P)], p16[:, ds(sblk * P, P)], ident[:]
                )
            nc.scalar.copy(pt[:, ds(g * 512, 512)], pst[:])

        # normalized fp32 attention rows out (before PV so the last tile's
        # attn store hides under the PV matmuls instead of extending the tail)
        nc.vector.tensor_scalar_mul(p32[:], p32[:], inv[:])
        nc.sync.dma_start(attn_d[ds(t * P, P), :], p32[:])

        # PV: ctx[t, h''] = sum_s P[t,s] v[s,h''] * inv
        for hc in range(HC):
            psc = ps_mm.tile([P, 512], F32, tag="mm")
            for si in range(ST):
                nc.tensor.matmul(
                    psc[:],
                    lhsT=pt[:, ds(si * P, P)],
                    rhs=v[:, si, ds(hc * 512, 512)],
                    start=(si == 0),
                    stop=(si == ST - 1),
                )
            cxs = p_cx.tile([P, 512], F32, tag="cxs")
            nc.vector.tensor_scalar_mul(cxs[:], psc[:], inv[:])
            nc.sync.dma_start(ctx_d[ds(t * P, P), ds(hc * 512, 512)], cxs[:])

    pending = emit_scores(0)
    for t in range(TT):
        nxt = emit_scores(t + 1) if t + 1 < TT else None
        emit_tail(t, *pending)
        pending = nxt


def build(repeat: int = 1, pair_split: bool | None = None) -> "bacc.Bacc":
    if pair_split is None:
        pair_split = PAIR_SPLIT
    nc = bacc.Bacc(
        "TRN2",
        target_bir_lowering=False,
        debug=False,
        enable_asserts=False,
        num_devices=NCORES,
    )
    s_in = TSH if pair_split else S  # v activations arrive halved if split
    srcT = nc.dram_tensor("srcT", [H, S], BF16, kind="ExternalInput").ap()
    tarT = nc.dram_tensor("tarT", [H, TSH], BF16, kind="ExternalInput").ap()
    oriT = nc.dram_tensor("oriT", [H, s_in], BF16, kind="ExternalInput").ap()
    wqT = nc.dram_tensor("wqT", [H, H], BF16, kind="ExternalInput").ap()
    wkT = nc.dram_tensor("wkT", [H, H], BF16, kind="ExternalInput").ap()
    wvT = nc.dram_tensor("wvT", [H, H], BF16, kind="ExternalInput").ap()
    ctx_d = nc.dram_tensor("ctx", [TSH, H], F32, kind="ExternalOutput").ap()
    attn_d = nc.dram_tensor("attn", [TSH, S], F32, kind="ExternalOutput").ap()

    # [h, x] -> [p, ho, x] with h = ho*P + p (partition-inner)
    srcT_r = srcT.rearrange("(ko p) s -> p ko s", p=P)
    tarT_r = tarT.rearrange("(ko p) t -> p ko t", p=P)
    oriT_r = oriT.rearrange("(ko p) s -> p ko s", p=P)
    wqT_r = wqT.rearrange("(ko p) m -> p ko m", p=P)
    wkT_r = wkT.rearrange("(ko p) m -> p ko m", p=P)
    wvT_r = wvT.rearrange("(ko p) m -> p ko m", p=P)

    with tile.TileContext(nc) as tc:
        with (
            tc.tile_pool(name="w", bufs=2) as p_w,
            tc.tile_pool(name="act", bufs=3) as p_act,
            tc.tile_pool(name="qkv", bufs=1) as p_qkv,
            tc.tile_pool(name="p32", bufs=2) as p_p32,
            tc.tile_pool(name="p16", bufs=2) as p_p16,
            tc.tile_pool(name="pt", bufs=2) as p_pt,
            tc.tile_pool(name="stat", bufs=3) as p_stat,
            tc.tile_pool(name="cxs", bufs=3) as p_cx,
            tc.tile_pool(name="cst", bufs=1) as p_cst,
            tc.tile_pool(name="dram", bufs=1, space="DRAM") as p_dram,
            tc.tile_pool(name="mm", bufs=6, space="PSUM") as ps_mm,
            tc.tile_pool(name="tp", bufs=2, space="PSUM") as ps_tp,
        ):
            ident = p_cst.tile([P, P], BF16, tag="ident")
            make_identity(nc, ident[:])
            pools = (
                p_w, p_act, p_qkv, p_p32, p_p16, p_pt, p_stat, p_cx, p_dram,
                ps_mm, ps_tp,
            )
            aps = (srcT_r, tarT_r, oriT_r, wqT_r, wkT_r, wvT_r, ctx_d, attn_d, ident)
            for _ in range(repeat):
                _emit_body(tc, pools, aps, pair_split)
    nc.compile()
    return nc


def get_nc(repeat: int = 1, pair_split: bool | None = None) -> "bacc.Bacc":
    if pair_split is None:
        pair_split = PAIR_SPLIT
    key = (repeat, pair_split)
    if key not in _NC_CACHE:
        _NC_CACHE[key] = build(repeat, pair_split)
    return _NC_CACHE[key]


def shard_inputs(tar, src, ori_src, Wq, Wk, Wv):
    """Host-side layout work only: transpose / cast / slice (no math)."""
    bf = ml_dtypes.bfloat16
    f32 = np.float32
    tar = np.asarray(tar, f32)
    src = np.asarray(src, f32)
    ori_src = np.asarray(ori_src, f32)
    wqT = np.ascontiguousarray(np.asarray(Wq, f32).T).astype(bf)
    wkT = np.ascontiguousarray(np.asarray(Wk, f32).T).astype(bf)
    wvT = np.ascontiguousarray(np.asarray(Wv, f32).T).astype(bf)
    srcT = np.ascontiguousarray(src.transpose(0, 2, 1)).astype(bf)  # [B, H, S]
    oriT = np.ascontiguousarray(ori_src.transpose(0, 2, 1)).astype(bf)
    tarT = np.ascontiguousarray(tar.transpose(0, 2, 1)).astype(bf)  # [B, H, T]

    in_maps = []
    for c in range(NCORES):
        b, th = divmod(c, 2)
        sl = slice(th * TSH, (th + 1) * TSH)
        in_maps.append(
            {
                "srcT": srcT[b],
                "tarT": np.ascontiguousarray(tarT[b][:, sl]),
                "oriT": np.ascontiguousarray(oriT[b][:, sl]) if PAIR_SPLIT else oriT[b],
                "wqT": wqT,
                "wkT": wkT,
                "wvT": wvT,
            }
        )
    return in_maps


class Dispatcher:
    """Compile-once / run-many PJRT dispatch for one Bass program.

    Mirrors concourse.bass2jax.run_bass_via_pjrt but caches the jitted
    callable and keeps inputs resident on device, so repeated runs do not
    pay re-trace/re-compile or host->device transfer.
    """

    def __init__(self, nc):
        import jax
        from jax.sharding import Mesh, PartitionSpec
        from jax.experimental.shard_map import shard_map
        from concourse import bass2jax

        bass2jax.install_neuronx_cc_hook()
        self._jax = jax
        self.nc = nc
        in_names, out_names, out_avals, zero_outs = [], [], [], []
        partition_name = (
            nc.partition_id_tensor.name if nc.partition_id_tensor else None
        )
        for alloc in nc.m.functions[0].allocations:
            if not isinstance(alloc, mybir.MemoryLocationSet):
                continue
            name = alloc.memorylocations[0].name
            if alloc.kind == "ExternalInput":
                if name != partition_name:
                    in_names.append(name)
            elif alloc.kind == "ExternalOutput":
                shape = tuple(alloc.tensor_shape)
                dtype = mybir.dt.np(alloc.dtype)
                out_names.append(name)
                out_avals.append(jax.core.ShapedArray(shape, dtype))
                zero_outs.append(np.zeros(shape, dtype))
        self.in_names = list(in_names)
        self.out_names = out_names
        self.out_avals = out_avals
        self.zero_outs = zero_outs
        n_params = len(in_names)
        all_in_names = in_names + out_names
        if partition_name is not None:
            all_in_names.append(partition_name)

        from concourse.bass2jax import _bass_exec_p, partition_id_tensor

        def _body(*args):
            operands = list(args)
            if partition_name is not None:
                operands.append(partition_id_tensor())
            outs = _bass_exec_p.bind(
                *operands,
                out_avals=tuple(out_avals),
                in_names=tuple(all_in_names),
                out_names=tuple(out_names),
                lowering_input_output_aliases=(),
                sim_require_finite=True,
                sim_require_nnan=True,
                nc=nc,
            )
            return tuple(outs)

        devices = jax.devices()[:NCORES]
        mesh = Mesh(np.asarray(devices), ("core",))
        n_all = n_params + len(out_names)
        self._fn = jax.jit(
            shard_map(
                _body,
                mesh=mesh,
                in_specs=(PartitionSpec("core"),) * n_all,
                out_specs=(PartitionSpec("core"),) * len(out_names),
                check_rep=False,
            ),
            keep_unused=True,
        )
        self._dev_zero = None

    def put(self, in_maps):
        """Concat per-core inputs on axis 0 and move to device once."""
        jax = self._jax
        concat = [
            np.concatenate([np.asarray(m[name]) for m in in_maps], axis=0)
            for name in self.in_names
        ]
        dev_in = [jax.device_put(a) for a in concat]
        if self._dev_zero is None:
            self._dev_zero = [
                jax.device_put(
                    np.zeros((NCORES * z.shape[0], *z.shape[1:]), z.dtype)
                )
                for z in self.zero_outs
            ]
        return dev_in

    def run(self, dev_in):
        outs = self._fn(*dev_in, *self._dev_zero)
        self._jax.block_until_ready(outs)
        return outs

    def run_host(self, in_maps):
        outs = self.run(self.put(in_maps))
        per_core = []
        for c in range(NCORES):
            d = {}
            for i, name in enumerate(self.out_names):
                full = np.asarray(outs[i])
                per_shard = self.out_avals[i].shape
                d[name] = full.reshape(NCORES, *per_shard)[c]
            per_core.append(d)
        return per_core


def get_dispatcher(repeat: int = 1) -> Dispatcher:
    key = (repeat, PAIR_SPLIT)
    if key not in _JIT_CACHE:
        _JIT_CACHE[key] = Dispatcher(get_nc(repeat))
    return _JIT_CACHE[key]


def kernel(tar, src, ori_src, Wq, Wk, Wv):
    in_maps = shard_inputs(tar, src, ori_src, Wq, Wk, Wv)
    results = get_dispatcher().run_host(in_maps)
    context = np.empty((B, T, H), np.float32)
    attn = np.empty((B, T, S), np.float32)
    for c in range(NCORES):
        b, th = divmod(c, 2)
        context[b, th * TSH : (th + 1) * TSH] = results[c]["ctx"]
        attn[b, th * TSH : (th + 1) * TSH] = results[c]["attn"]
    return context, attn
